# revision 30
# baseline (speedup 1.0000x reference)
"""Trainium2 Bass kernel for the EngramNew module (dense_cnn).

Sharding: B*T = 8192 tokens split across 8 cores (1024 tokens each, the
conv halo of (K-1)*DIL = 9 tokens is precomputed host-side). On-device
layout is channels-on-partitions / tokens-on-free: [G*C, T_core].
"""

import sys

for _p in ("/opt/trn_rl_repo",):
    if _p not in sys.path:
        sys.path.insert(0, _p)

import numpy as np

import concourse.bass as bass
from concourse import mybir
from concourse.tile import TileContext
from concourse.vector_clock import ScopedClock
from concourse.bass_utils import run_bass_kernel_spmd
import bass_rust

F32 = mybir.dt.float32
F32R = mybir.dt.float32r
AF = mybir.ActivationFunctionType

# Problem constants (hardcoded per spec nn_EngramNew_2070174237244)
B, T, G, C, E = 2, 4096, 4, 1024, 1024
GC = G * C
KT, DIL = 4, 3          # conv taps / dilation
EPS = 1e-5
NORM_EPS = 1e-5
NCORES = 8
NTOK = (B * T) // NCORES    # 1024 tokens per core
HALO = (KT - 1) * DIL       # 9
NET = E // 128              # 8 e-tiles
NGCT = GC // 128            # 32 gc-tiles
NCT = C // 128              # 8 c-tiles
CHW = 512                   # token chunk width (1 PSUM bank of fp32)
NCH = NTOK // CHW           # 2 chunks


class PatchedTileContext(TileContext):
    """This walrus build allows only one sem wait per instruction (two on
    EventSemaphore). Tile attaches as many waits as an instruction needs,
    so after scheduling we hoist excess waits onto no-op instructions
    inserted just before the owner on the same engine (engines are strict
    FIFO, so observing the sems earlier is equivalent)."""

    def _split_excess_waits(self):
        nc = self.nc

        def make_nop(engine):
            bi = nc.engines[engine].nop()
            bb = nc.cur_bb.bb
            lst = bb.instructions
            assert lst[-1] is bi.ins
            bb.instructions = lst[:-1]
            return bi.ins

        for f in nc.m.functions:
            for blk in f.blocks:
                insts = blk.instructions
                out = []
                changed = False
                for ins in insts:
                    si = ins.sync_info
                    waits = list(si.on_wait) if (si and si.on_wait) else []
                    cap = 2 if isinstance(ins, mybir.InstEventSemaphore) else 1
                    if len(waits) > cap:
                        changed = True
                        for w in waits[cap:]:
                            nop = make_nop(ins.engine)
                            nop.sync_info = bass_rust.SyncInfo(
                                on_wait=[w], on_update=[]
                            )
                            out.append(nop)
                        upd = list(si.on_update) if si.on_update else []
                        ins.sync_info = bass_rust.SyncInfo(
                            on_wait=waits[:cap], on_update=upd
                        )
                    out.append(ins)
                if changed:
                    blk.instructions = out

    def _drain_and_barrier(self, tick_clock, wait_clock):
        super()._drain_and_barrier(tick_clock, wait_clock)
        self._split_excess_waits()


def _r(ap):
    return ap.bitcast(F32R)


def build_program():
    nc = bass.Bass()
    # register the float biases used by activation ops
    for cval in (float(C) * EPS, EPS):
        if (F32, cval) not in nc.const_aps.aps:
            t = nc.alloc_sbuf_tensor(f"const-float32-{cval}", [128, 1], F32)
            nc.gpsimd.memset(t.ap(), cval)
            nc.const_aps.aps[(F32, cval)] = t.ap()
    nc.all_engine_barrier()

    # ---- DRAM parameters (per-core shapes) ----
    embT = nc.declare_dram_parameter("embT", [E, NTOK], F32, isOutput=False)
    hidT = nc.declare_dram_parameter("hidT", [GC, NTOK], F32, isOutput=False)
    kwT = nc.declare_dram_parameter("kwT", [E, GC], F32, isOutput=False)
    vwT = nc.declare_dram_parameter("vwT", [E, C], F32, isOutput=False)
    keyb = nc.declare_dram_parameter("keyb", [128, NGCT], F32, isOutput=False)
    valb = nc.declare_dram_parameter("valb", [128, NCT], F32, isOutput=False)
    # masked lhsT columns for the partition-sum matmuls, [128, 16] each
    lhsT_k = nc.declare_dram_parameter("lhsT_k", [NGCT, 128, 16], F32, isOutput=False)
    lhsT_q = nc.declare_dram_parameter("lhsT_q", [NGCT, 128, 16], F32, isOutput=False)
    lhsT_kq = nc.declare_dram_parameter("lhsT_kq", [NGCT, 128, 16], F32, isOutput=False)
    lhsT_v = nc.declare_dram_parameter("lhsT_v", [NCT, 128, 16], F32, isOutput=False)
    # row-mover selectors [16, 4] per quantity
    selq = nc.declare_dram_parameter("selq", [16, 4 * 4], F32, isOutput=False)
    # broadcast lhsT: [4, 128] per group g (row g ones)
    bcast = nc.declare_dram_parameter("bcast", [4, 4 * 128], F32, isOutput=False)
    # folded conv weights (w_norm folded in): column gct*4+k is per-partition tap
    cwf = nc.declare_dram_parameter("cwf", [128, NGCT * KT], F32, isOutput=False)
    ident = nc.declare_dram_parameter("ident", [128, 128], F32, isOutput=False)
    halo = nc.declare_dram_parameter("halo", [GC, HALO], F32, isOutput=False)
    out_d = nc.declare_dram_parameter("out", [GC, NTOK], F32, isOutput=True)

    with PatchedTileContext(nc) as tc:
        consts = tc.alloc_tile_pool(name="consts", bufs=1)
        emb_all = consts.tile([128, NET, NTOK], F32R)
        for et in range(NET):
            nc.sync.dma_start(
                out=emb_all[:, et, :],
                in_=_r(embT[et * 128:(et + 1) * 128, :]),
            )
        vproj_all = consts.tile([128, NCT, NTOK], F32)
        keyb_sb = consts.tile([128, NGCT], F32)
        nc.sync.dma_start(out=keyb_sb, in_=keyb[:, :])
        valb_sb = consts.tile([128, NCT], F32)
        nc.sync.dma_start(out=valb_sb, in_=valb[:, :])
        lk_sb = consts.tile([128, NGCT, 16], F32R)
        nc.sync.dma_start(out=lk_sb, in_=_r(lhsT_k.rearrange("n p m -> p n m")))
        lq_sb = consts.tile([128, NGCT, 16], F32R)
        nc.sync.dma_start(out=lq_sb, in_=_r(lhsT_q.rearrange("n p m -> p n m")))
        lkq_sb = consts.tile([128, NGCT, 16], F32R)
        nc.sync.dma_start(out=lkq_sb, in_=_r(lhsT_kq.rearrange("n p m -> p n m")))
        lv_sb = consts.tile([128, NCT, 16], F32R)
        nc.sync.dma_start(out=lv_sb, in_=_r(lhsT_v.rearrange("n p m -> p n m")))
        selq_sb = consts.tile([16, 4 * 4], F32R)
        nc.sync.dma_start(out=selq_sb, in_=_r(selq[:, :]))
        bcast_sb = consts.tile([4, 4 * 128], F32R)
        nc.sync.dma_start(out=bcast_sb, in_=_r(bcast[:, :]))
        cwf_sb = consts.tile([128, NGCT * KT], F32)
        nc.sync.dma_start(out=cwf_sb, in_=cwf[:, :])
        id_sb = consts.tile([128, 128], F32)
        nc.sync.dma_start(out=id_sb, in_=ident[:, :])

        rowm = tc.alloc_tile_pool(name="rowm", bufs=1)
        mmp = tc.alloc_tile_pool(name="mmpsum", bufs=3, space=bass.MemorySpace.PSUM)
        sump = tc.alloc_tile_pool(name="sumpsum", bufs=1, space=bass.MemorySpace.PSUM)
        wpool = tc.alloc_tile_pool(name="wstream", bufs=3)
        qpool = tc.alloc_tile_pool(name="qstream", bufs=2)
        scr = tc.alloc_tile_pool(name="scr", bufs=8)
        if True:
            sums = sump.tile([16, NTOK], F32)
            first_sum_mm = [True, True]  # per chunk bank

            def sum_mm(lhsT, rhs_tile, ch, last=False):
                st = first_sum_mm[ch]
                first_sum_mm[ch] = False
                nc.tensor.matmul(
                    sums[:, ch * CHW:(ch + 1) * CHW],
                    _r(lhsT),
                    _r(rhs_tile[:, ch * CHW:(ch + 1) * CHW]),
                    start=st,
                    stop=last,
                    skip_group_check=True,
                )

            # ---------- stage B: vproj = value_w @ emb + value_b ----------
            for ct in range(NCT):
                wv = wpool.tile([128, NET, 128], F32R, tag="wtile")
                nc.sync.dma_start(
                    out=wv,
                    in_=_r(vwT.rearrange("(et p) c -> p et c", p=128)[
                        :, :, ct * 128:(ct + 1) * 128
                    ]),
                )
                ps = mmp.tile([128, NTOK], F32, tag="mm")
                for ch in range(NCH):
                    for et in range(NET):
                        nc.tensor.matmul(
                            ps[:, ch * CHW:(ch + 1) * CHW],
                            _r(wv[:, et, :]),
                            _r(emb_all[:, et, ch * CHW:(ch + 1) * CHW]),
                            start=(et == 0),
                            stop=(et == NET - 1),
                        )
                vsq = scr.tile([128, NTOK], F32R, tag="sq")
                nc.scalar.activation(
                    vsq, ps, AF.Square, bias=valb_sb[:, ct:ct + 1], scale=1.0
                )
                nc.scalar.activation(
                    vproj_all[:, ct, :], ps, AF.Identity,
                    bias=valb_sb[:, ct:ct + 1], scale=1.0,
                )
                for ch in range(NCH):
                    sum_mm(lv_sb[:, ct, :], vsq, ch)

            # ---------- stage C: k path + gate reductions ----------
            def _emit_colsums(cg, cksq, cqsq, ckq, last):
                for ch in range(NCH):
                    sum_mm(lk_sb[:, cg, :], cksq, ch)
                    sum_mm(lq_sb[:, cg, :], cqsq, ch)
                    sum_mm(lkq_sb[:, cg, :], ckq, ch, last=last)

            pending = []
            for gct in range(NGCT):
                wk = wpool.tile([128, NET, 128], F32R, tag="wtile")
                nc.sync.dma_start(
                    out=wk,
                    in_=_r(kwT.rearrange("(et p) g -> p et g", p=128)[
                        :, :, gct * 128:(gct + 1) * 128
                    ]),
                )
                q_sb = qpool.tile([128, NTOK], F32, tag="q")
                nc.sync.dma_start(
                    out=q_sb, in_=hidT[gct * 128:(gct + 1) * 128, :]
                )
                ps = mmp.tile([128, NTOK], F32, tag="mm")
                for ch in range(NCH):
                    for et in range(NET):
                        nc.tensor.matmul(
                            ps[:, ch * CHW:(ch + 1) * CHW],
                            _r(wk[:, et, :]),
                            _r(emb_all[:, et, ch * CHW:(ch + 1) * CHW]),
                            start=(et == 0),
                            stop=(et == NET - 1),
                        )
                ksq = scr.tile([128, NTOK], F32R, tag="sq")
                nc.scalar.activation(
                    ksq, ps, AF.Square, bias=keyb_sb[:, gct:gct + 1], scale=1.0
                )
                qsq = scr.tile([128, NTOK], F32R, tag="sq")
                nc.scalar.activation(qsq, q_sb, AF.Square)
                kq = scr.tile([128, NTOK], F32R, tag="sq")
                # kq = (k_psum + key_b) * q in one fused DVE op (no eviction)
                nc.vector.scalar_tensor_tensor(
                    kq, ps, keyb_sb[:, gct:gct + 1], q_sb,
                    op0=mybir.AluOpType.add, op1=mybir.AluOpType.mult,
                )
                pending.append((gct, ksq, qsq, kq))
                if len(pending) > 2:
                    _emit_colsums(*pending.pop(0), last=False)
            while pending:
                _emit_colsums(*pending.pop(0), last=(not pending))

            # ---------- stage D: row math -> gate[4, NTOK], alpha[4, NTOK] ----------
            sums_sb = rowm.tile([16, NTOK], F32R)
            nc.scalar.activation(sums_sb, sums, AF.Copy)
            # move quantity rows to partitions 0..3 (via masked K=16 matmuls)
            qrows = []
            for qi in range(4):
                qt = rowm.tile([4, NTOK], F32, tag=f"qrow{qi}")
                for ch in range(NCH):
                    ap = mmp.tile([4, CHW], F32, tag="mm")
                    nc.tensor.matmul(
                        ap,
                        _r(selq_sb[:, qi * 4:(qi + 1) * 4]),
                        _r(sums_sb[:, ch * CHW:(ch + 1) * CHW]),
                        start=True,
                        stop=True,
                    )
                    if qi == 0:   # sk -> sk + C*EPS
                        nc.scalar.activation(
                            qt[:, ch * CHW:(ch + 1) * CHW], ap, AF.Identity,
                            bias=float(C) * EPS, scale=1.0,
                        )
                    elif qi == 1:  # sq -> sq/C + EPS
                        nc.scalar.activation(
                            qt[:, ch * CHW:(ch + 1) * CHW], ap, AF.Identity,
                            bias=EPS, scale=1.0 / float(C),
                        )
                    else:
                        nc.scalar.activation(
                            qt[:, ch * CHW:(ch + 1) * CHW], ap, AF.Copy
                        )
                qrows.append(qt)
            ak, aq, dot, sv = qrows

            # 1/sqrt(x) and sqrt(x) via exp(±0.5*ln(x)) — one ACT table set,
            # better ULP than the Sqrt/Rsqrt tables.
            p4 = rowm.tile([4, NTOK], F32)
            nc.vector.tensor_mul(p4, ak, aq)          # (sk/C+e)(sq/C+e)*C
            lnp = rowm.tile([4, NTOK], F32)
            nc.scalar.activation(lnp, p4, AF.Ln)
            r4 = rowm.tile([4, NTOK], F32)
            nc.scalar.activation(r4, lnp, AF.Exp, scale=-0.5)
            graw = rowm.tile([4, NTOK], F32)
            nc.vector.tensor_mul(graw, dot, r4)
            sgn = rowm.tile([4, NTOK], F32)
            nc.scalar.activation(sgn, graw, AF.Sign)
            ab4 = rowm.tile([4, NTOK], F32)
            nc.scalar.activation(ab4, graw, AF.Abs)
            nc.vector.tensor_scalar_max(ab4, ab4, 1e-6)
            lnc = rowm.tile([4, NTOK], F32)
            nc.scalar.activation(lnc, ab4, AF.Ln)
            sq4 = rowm.tile([4, NTOK], F32)
            nc.scalar.activation(sq4, lnc, AF.Exp, scale=0.5)
            ss4 = rowm.tile([4, NTOK], F32)
            nc.vector.tensor_mul(ss4, sq4, sgn)
            gate = rowm.tile([4, NTOK], F32R)
            nc.scalar.activation(gate, ss4, AF.Sigmoid)
            g2 = rowm.tile([4, NTOK], F32)
            nc.scalar.activation(g2, gate, AF.Square)
            gv = rowm.tile([4, NTOK], F32)
            nc.vector.tensor_mul(gv, g2, sv)
            lnv = rowm.tile([4, NTOK], F32)
            nc.scalar.activation(
                lnv, gv, AF.Ln, bias=NORM_EPS, scale=1.0 / float(C)
            )
            rv4 = rowm.tile([4, NTOK], F32)
            nc.scalar.activation(rv4, lnv, AF.Exp, scale=-0.5)
            alpha = rowm.tile([4, NTOK], F32R)
            nc.vector.tensor_mul(alpha, gate, rv4)

        # ---------- stage E: value, normed, conv, silu, residual ----------
        for p in (scr, qpool, wpool, sump, mmp):
            p.release()
        bcp = tc.alloc_tile_pool(name="bcpsum", bufs=1, space=bass.MemorySpace.PSUM)
        accp = tc.alloc_tile_pool(name="accpsum", bufs=2, space=bass.MemorySpace.PSUM)
        npool = tc.alloc_tile_pool(name="nbuf", bufs=3)
        opool = tc.alloc_tile_pool(name="obuf", bufs=2)
        vpool = tc.alloc_tile_pool(name="vbuf", bufs=4)
        dpool = tc.alloc_tile_pool(name="dbuf", bufs=3)
        if True:
            def _emit_tail(tgct, tnx, tdg, tval):
                acc = accp.tile([128, NTOK], F32, tag="acc")
                for ch in range(NCH):
                    for k in range(KT):
                        nc.tensor.matmul(
                            acc[:, ch * CHW:(ch + 1) * CHW],
                            _r(tdg[:, k * 128:(k + 1) * 128]),
                            _r(tnx[:, ch * CHW + k * DIL:ch * CHW + k * DIL + CHW]),
                            start=(k == 0),
                            stop=(k == KT - 1),
                        )
                sacc = opool.tile([128, NTOK], F32, tag="sacc")
                nc.scalar.activation(sacc, acc, AF.Silu)
                ot = opool.tile([128, NTOK], F32, tag="ot")
                nc.gpsimd.tensor_add(ot, tval, sacc)
                nc.sync.dma_start(
                    out=out_d[tgct * 128:(tgct + 1) * 128, :], in_=ot
                )

            epending = None
            for g in range(G):
                gb = bcp.tile([128, NTOK], F32, tag="gb")
                ab = bcp.tile([128, NTOK], F32, tag="ab")
                for ch in range(NCH):
                    nc.tensor.matmul(
                        gb[:, ch * CHW:(ch + 1) * CHW],
                        _r(bcast_sb[:, g * 128:(g + 1) * 128]),
                        _r(gate[:, ch * CHW:(ch + 1) * CHW]),
                        start=True, stop=True,
                    )
                    nc.tensor.matmul(
                        ab[:, ch * CHW:(ch + 1) * CHW],
                        _r(bcast_sb[:, g * 128:(g + 1) * 128]),
                        _r(alpha[:, ch * CHW:(ch + 1) * CHW]),
                        start=True, stop=True,
                    )
                for ct in range(NCT):
                    gct = g * NCT + ct
                    nx = npool.tile([128, HALO + NTOK], F32R, tag="nx")
                    nc.sync.dma_start(
                        out=nx[:, 0:HALO],
                        in_=_r(halo[gct * 128:(gct + 1) * 128, :]),
                    )
                    nc.vector.tensor_mul(
                        nx[:, HALO:HALO + NTOK], vproj_all[:, ct, :], ab
                    )
                    val = vpool.tile([128, NTOK], F32, tag="val")
                    nc.vector.tensor_mul(val, vproj_all[:, ct, :], gb)
                    dg = dpool.tile([128, KT * 128], F32R, tag="dg")
                    for k in range(KT):
                        nc.scalar.mul(
                            dg[:, k * 128:(k + 1) * 128],
                            id_sb,
                            cwf_sb[:, gct * KT + k:gct * KT + k + 1],
                        )
                    if epending is not None:
                        _emit_tail(*epending)
                    epending = (gct, nx, dg, val)
            _emit_tail(*epending)
        for p in (dpool, vpool, opool, npool, accp, bcp, rowm, consts):
            p.release()
    return nc


def host_prep(embeddings, hidden_states, key_w, key_b, value_w, value_b,
              w_key_norm, w_query_norm, w_norm, conv_weight):
    """Build the per-core input maps (all f32 numpy)."""
    f32 = np.float32
    embeddings = np.asarray(embeddings, f32)
    hidden_states = np.asarray(hidden_states, f32)
    key_w = np.asarray(key_w, f32)
    key_b = np.asarray(key_b, f32)
    value_w = np.asarray(value_w, f32)
    value_b = np.asarray(value_b, f32)
    w_key_norm = np.asarray(w_key_norm, f32)
    w_query_norm = np.asarray(w_query_norm, f32)
    w_norm = np.asarray(w_norm, f32)
    conv_weight = np.asarray(conv_weight, f32)

    kwT = np.ascontiguousarray(key_w.T)                    # [E, GC]
    vwT = np.ascontiguousarray(value_w.T)                  # [E, C]
    keyb_r = np.ascontiguousarray(key_b.reshape(NGCT, 128).T)  # [128, NGCT]
    valb_r = np.ascontiguousarray(value_b.reshape(NCT, 128).T)
    wkq = (w_key_norm * w_query_norm).reshape(GC)

    lk = np.zeros((NGCT, 128, 16), f32)
    lq = np.zeros((NGCT, 128, 16), f32)
    lkq = np.zeros((NGCT, 128, 16), f32)
    for gct in range(NGCT):
        g = gct // NCT
        lk[gct, :, g] = 1.0
        lq[gct, :, 4 + g] = 1.0
        lkq[gct, :, 8 + g] = wkq[gct * 128:(gct + 1) * 128]
    lv = np.zeros((NCT, 128, 16), f32)
    lv[:, :, 12:16] = 1.0

    selq = np.zeros((16, 16), f32)
    for qi in range(4):
        for g in range(4):
            selq[qi * 4 + g, qi * 4 + g] = 1.0
    bcast = np.zeros((4, 4 * 128), f32)
    for g in range(4):
        bcast[g, g * 128:(g + 1) * 128] = 1.0

    cwf = (conv_weight.reshape(G, C, KT) * w_norm[:, :, None]).astype(f32)
    cwf_r = np.zeros((128, NGCT * KT), f32)
    for gct in range(NGCT):
        g, ct = gct // NCT, gct % NCT
        for k in range(KT):
            cwf_r[:, gct * KT + k] = cwf[g, ct * 128:(ct + 1) * 128, k]
    ident = np.eye(128, dtype=f32)

    in_maps = []
    for core in range(NCORES):
        b = core // (NCORES // B)
        t0 = (core % (NCORES // B)) * NTOK
        emb_s = embeddings[b, t0:t0 + NTOK]                # [NTOK, E]
        hid_s = hidden_states[b, t0:t0 + NTOK].reshape(NTOK, GC)
        embT_c = np.ascontiguousarray(emb_s.T)             # [E, NTOK]
        hidT_c = np.ascontiguousarray(hid_s.T)             # [GC, NTOK]

        # halo: nhat (= value / rms_v, w_norm NOT applied) for the 9
        # preceding tokens; zeros at the sequence start.
        if t0 == 0:
            halo_c = np.zeros((GC, HALO), f32)
        else:
            th = slice(t0 - HALO, t0)
            e9 = embeddings[b, th]                          # [9, E]
            k9 = (e9 @ key_w.T + key_b).reshape(HALO, G, C)
            q9 = hidden_states[b, th]                       # [9, G, C]
            rk = np.sqrt((k9 * k9).mean(-1) + EPS)
            rq = np.sqrt((q9 * q9).mean(-1) + EPS)
            d9 = np.einsum("tgc,gc,tgc,gc->tg", k9, w_key_norm, q9, w_query_norm)
            graw = d9 / (rk * rq) / np.sqrt(f32(C))
            g9 = 1.0 / (1.0 + np.exp(-(np.where(graw >= 0, 1.0, -1.0)
                                       * np.sqrt(np.maximum(np.abs(graw), 1e-6)))))
            vp9 = e9 @ value_w.T + value_b                  # [9, C]
            val9 = vp9[:, None, :] * g9[..., None].astype(f32)
            rv9 = np.sqrt((val9 * val9).mean(-1) + NORM_EPS)
            nhat9 = val9 / rv9[..., None]
            halo_c = np.ascontiguousarray(
                nhat9.transpose(1, 2, 0).reshape(GC, HALO).astype(f32))

        in_maps.append({
            "embT": embT_c, "hidT": hidT_c, "kwT": kwT, "vwT": vwT,
            "keyb": keyb_r, "valb": valb_r,
            "lhsT_k": lk, "lhsT_q": lq, "lhsT_kq": lkq, "lhsT_v": lv,
            "selq": selq, "bcast": bcast, "cwf": cwf_r, "ident": ident,
            "halo": halo_c,
        })
    return in_maps


_NC_CACHE = [None]
LAST_RESULT = [None]


def kernel(**inputs) -> np.ndarray:
    in_maps = host_prep(**inputs)
    if _NC_CACHE[0] is None:
        _NC_CACHE[0] = build_program()
    nc = _NC_CACHE[0]
    res = run_bass_kernel_spmd(nc, in_maps, list(range(NCORES)))
    LAST_RESULT[0] = res
    out = np.empty((B, T, G, C), np.float32)
    for core in range(NCORES):
        b = core // (NCORES // B)
        t0 = (core % (NCORES // B)) * NTOK
        oc = res.results[core]["out"]                      # [GC, NTOK]
        out[b, t0:t0 + NTOK] = oc.reshape(G, C, NTOK).transpose(2, 0, 1)
    return out
